# revision 12
# baseline (speedup 1.0000x reference)
"""CORAL focal multi-task loss on 8 Trainium2 NeuronCores.

Data-parallel: the 2M-row batch is split into 8 shards of 250k rows
(padded to 128*1954). Each core computes PSUM partial sums of the
weighted focal-CORAL loss elements for the 3 tasks; the host sums the
8 x 3 x 512 partials and normalizes.

Math. For one element with logit x, ordinal bit b = (t > c), kl weight w:
  loss_elem = w * (0.75 - 0.5 b) * Fc((1-2b) * x),
  Fc(z) = sigmoid(z)^2 * softplus(z)
since  -log(sigmoid(z)) = softplus(-z)  and  1 - sigmoid(z) = sigmoid(-z).
Fc is evaluated in ONE ScalarE pass via a custom activation table (the
`gelu` slot of the gelu_and_others set is rewritten with Taylor cubics of
Fc at the stock bucket centers; see _ensure_actroot / work/mktable.py).

Device pipeline per tile ([128, 10*Q] fp16 column slabs):
  DVE: smh = (t <= c) - 0.5            (per-column tensor_scalar)
       y'  = x * smh                    (= +-x/2)
       a   = ACT Fc(2*y')               (ScalarE, custom table)
       ab  = 0.5*smh + 0.5              (= 0.25 / 0.75)
       abw = ab * w_row (broadcast AP)
       wet = a * abw
  PE:  ones^T @ wet chunks accumulate into per-task PSUM [1,512].
  w_row = sum_c (kl_t == c) * cw[c]; padding rows carry kl_t = 7 -> w = 0.
"""

import json
import os
import shutil
import numpy as np

import concourse.bacc as bacc
import concourse.mybir as mybir
import concourse.tile as tile
from concourse.bass_utils import run_bass_kernel_spmd

AluOp = mybir.AluOpType
ActFn = mybir.ActivationFunctionType
dt = mybir.dt

EPS = 1e-8
N = 2_000_000
NCORES = 8
NCORE = N // NCORES            # 250_000 rows per core
RPP = 1954                     # rows per partition (128*1954 = 250_112)
NPAD = 128 * RPP               # padded rows per core
TILE_Q = [490, 490, 490, 484]  # rows-per-partition per tile (sum = RPP)
NCOLS = 10                     # 4 kl + 3 jsnm + 3 jsnl
TASKS = [(0, 4), (4, 3), (7, 3)]  # (column offset, n columns)

ACTROOT = os.path.join(os.path.dirname(os.path.abspath(__file__)), "actroot")

_CACHED = {}


# ---------------------------------------------------------------------------
# Custom activation table: rewrite the `gelu` buckets of gelu_and_others so
# that ActivationFunctionType.Gelu evaluates Fc(z) = sigmoid(z)^2*softplus(z).
# Bucket entry format (32B): [d0, d1, d2, d3, x0, 0, 0, 0] — Taylor coeffs
# around x0. Entries 0..503 are gelu's dense buckets, 504/505 small-signal,
# 506 positive saturation, 507 negative saturation.
# ---------------------------------------------------------------------------

def _fc_taylor_coeffs(x0s):
    """Taylor coefficients [F, F', F''/2, F'''/6] of Fc at each x0 (float64)."""
    x = np.asarray(x0s, dtype=np.float64)
    u = 1.0 / (1.0 + np.exp(-x))
    sp = np.logaddexp(0.0, x)
    up = u * (1 - u)
    F = u * u * sp
    A = 2 * (1 - u) * sp + u
    F1 = u * u * A
    Ap = up * (1 - 2 * sp) + 2 * u * (1 - u)
    F2 = 2 * u * up * A + u * u * Ap
    # third derivative numerically (Richardson) — plenty accurate in f64
    h = 1e-4

    def F2f(xx):
        uu = 1.0 / (1.0 + np.exp(-xx))
        ssp = np.logaddexp(0.0, xx)
        uup = uu * (1 - uu)
        AA = 2 * (1 - uu) * ssp + uu
        AAp = uup * (1 - 2 * ssp) + 2 * uu * (1 - uu)
        return 2 * uu * uup * AA + uu * uu * AAp

    F3 = (F2f(x + h) - F2f(x - h)) / (2 * h)
    return F, F1, F2 / 2.0, F3 / 6.0


def _ensure_actroot():
    """Build ACTROOT (idempotent) from the stock pwp_bin_trainium dir."""
    marker = os.path.join(ACTROOT, ".fc_table_v2")
    if os.path.exists(marker):
        return
    from neuronxcc.driver.Job import Job
    from neuronxcc.driver.jobs.support.FindActInfo import findActInfoFile

    src = os.path.dirname(findActInfoFile(Job.getPackageDir(), "gen3"))
    os.makedirs(ACTROOT, exist_ok=True)
    for f in os.listdir(src):
        shutil.copy(os.path.join(src, f), os.path.join(ACTROOT, f))

    bkt_path = os.path.join(ACTROOT, "gelu_and_others_bkt.bin")
    e = np.frombuffer(open(bkt_path, "rb").read(),
                      dtype=np.float32).reshape(-1, 8).copy()
    x0 = e[:504, 4].astype(np.float64)
    d0, d1, d2, d3 = _fc_taylor_coeffs(x0)
    e[:504, 0] = d0
    e[:504, 1] = d1
    e[:504, 2] = d2
    e[:504, 3] = d3
    t0 = [c[0] for c in _fc_taylor_coeffs(np.array([0.0]))]
    e[504, 0:4] = t0
    e[504, 4] = 0.0
    e[505, 0:4] = t0
    e[505, 4] = 0.0
    # positive tail (z > ~4.918): cubic least-squares fit around 0
    zs = np.linspace(4.9185, 10.0, 400)
    u = 1.0 / (1.0 + np.exp(-zs))
    ys = u * u * np.logaddexp(0.0, zs)
    A = np.vstack([np.ones_like(zs), zs, zs**2, zs**3]).T
    c, *_ = np.linalg.lstsq(A, ys, rcond=None)
    e[506, 0:4] = c
    e[506, 4] = 0.0
    e[507, 0:4] = 0.0
    e[507, 4] = 0.0
    open(bkt_path, "wb").write(e.tobytes())

    pj_path = os.path.join(ACTROOT, "gelu_and_others.json")
    pj = json.load(open(pj_path))
    fz = int(np.float32(float(d0[0]) * 0 + 0.17328679513998632).view(np.uint32))
    for ent in pj["profile_meta_data"]:
        if ent["func_name"] == "gelu_4p":
            ent["fzero_result"] = fz
            ent["fpinf_result"] = int(np.float32(3.4028235e38).view(np.uint32))
            ent["fninf_result"] = 0
    json.dump(pj, open(pj_path, "w"), indent=1)
    open(marker, "w").write("ok")


def _build_nc(rep=1):
    nc = bacc.Bacc("TRN2", num_devices=NCORES)

    xb = nc.dram_tensor("xb", [NPAD * NCOLS], dt.float16, kind="ExternalInput")
    tb = nc.dram_tensor("tb", [NPAD * 3], dt.float16, kind="ExternalInput")
    cw = nc.dram_tensor("cw", [1, 5], dt.float32, kind="ExternalInput")
    po = nc.dram_tensor("po", [1, 6 * 512], dt.float32, kind="ExternalOutput")

    with tile.TileContext(nc) as tc:
        with (
            tc.tile_pool(name="singles", bufs=1) as singles,
            tc.tile_pool(name="io", bufs=3) as io,
            tc.tile_pool(name="work", bufs=2) as wk,
            tc.tile_pool(name="ps", bufs=1, space="PSUM") as psp,
        ):
            cwt = singles.tile([128, 5], dt.float32)
            nc.sync.dma_start(out=cwt[:], in_=cw[:].to_broadcast((128, 5)))
            ones16 = singles.tile([128, 1], dt.float16)
            nc.vector.memset(ones16[:], 1.0)

            pss = [psp.tile([1, 512], dt.float32, tag=f"ps{t}", name=f"ps{t}")
                   for t in range(6)]
            started = [False] * 6

            # total matmul chunks per task (to set stop on the last one)
            def chunks_for(Q, C):
                L = C * Q
                out = []
                off = 0
                while off < L:
                    ln = min(512, L - off)
                    out.append((off, ln))
                    off += ln
                return out

            total_mm = [sum(len(chunks_for(Q, C)) for Q in TILE_Q)
                        for _, C in TASKS] * 2
            done_mm = [0] * 6

            import contextlib
            loop_ctx = (tc.For_i(0, rep, 1, hint_engines=(
                mybir.EngineType.DVE, mybir.EngineType.Activation,
                mybir.EngineType.SP, mybir.EngineType.PE)) if rep > 1
                else contextlib.nullcontext())
            with loop_ctx:
                base_x = 0
                base_t = 0
                for j, Q in enumerate(TILE_Q):
                    F = NCOLS * Q
                    xt = io.tile([128, F], dt.float16, tag="xt")
                    nc.sync.dma_start(
                        out=xt[:],
                        in_=xb[base_x:base_x + 128 * F].rearrange(
                            "(p f) -> p f", p=128))
                    tt = io.tile([128, 3 * Q], dt.float16, tag="tt")
                    nc.sync.dma_start(
                        out=tt[:],
                        in_=tb[base_t:base_t + 128 * 3 * Q].rearrange(
                            "(p f) -> p f", p=128))
                    base_x += 128 * F
                    base_t += 128 * 3 * Q

                    # smh = (t <= c) - 0.5   (per ordinal column)
                    smh = wk.tile([128, F], dt.float16, tag="smh")
                    ts_eng = nc.gpsimd if os.environ.get("KGPS") else nc.vector
                    col = 0
                    for tsk, (coff, C) in enumerate(TASKS):
                        tslab = tt[:, tsk * Q:(tsk + 1) * Q]
                        for c in range(C):
                            ts_eng.tensor_scalar(
                                smh[:, col * Q:(col + 1) * Q], tslab,
                                float(c), 0.5, AluOp.is_le, AluOp.subtract)
                            col += 1

                    # y' = x * smh ; a = Fc(2 y')
                    yt = wk.tile([128, F], dt.float16, tag="yt")
                    nc.vector.tensor_mul(yt[:], xt[:], smh[:])
                    at = wk.tile([128, F], dt.float16, tag="at")
                    nc.scalar.activation(at[:], yt[:], ActFn.Gelu, scale=2.0)

                    # per-row weight from kl_t
                    wr = wk.tile([128, Q], dt.float16, tag="wr")
                    tmpw = wk.tile([128, Q], dt.float16, tag="tmpw")
                    tkl = tt[:, 0:Q]
                    for c in range(5):
                        dst = wr if c == 0 else tmpw
                        nc.vector.tensor_scalar(
                            dst[:], tkl, float(c), cwt[:, c:c + 1],
                            AluOp.is_equal, AluOp.mult)
                        if c > 0:
                            nc.vector.tensor_add(wr[:], wr[:], tmpw[:])

                    # wa = a * w_row ; q = smh * wa
                    # sum(ab*w*a) = 0.5*sum(wa) + 0.5*sum(q)
                    wa = wk.tile([128, F], dt.float16, tag="wa")
                    wr_b = wr[:].rearrange("p (o q) -> p o q", o=1).to_broadcast(
                        (128, NCOLS, Q))
                    nc.vector.tensor_tensor(
                        wa[:].rearrange("p (o q) -> p o q", o=NCOLS),
                        at[:].rearrange("p (o q) -> p o q", o=NCOLS),
                        wr_b, AluOp.mult)
                    qt = wk.tile([128, F], dt.float16, tag="qt")
                    nc.vector.tensor_mul(qt[:], smh[:], wa[:])

                    # PE reduction: ones^T @ {wa, q} chunks -> PSUM accumulate
                    for half, src in ((0, wa), (1, qt)):
                        for t, (coff, C) in enumerate(TASKS):
                            k = half * 3 + t
                            for (off, ln) in chunks_for(Q, C):
                                first = not started[k]
                                done_mm[k] += 1
                                last = done_mm[k] == total_mm[k]
                                nc.tensor.matmul(
                                    pss[k][0:1, 0:ln], ones16[:, 0:1],
                                    src[:, coff * Q + off: coff * Q + off + ln],
                                    start=first, stop=last)
                                started[k] = True

            outt = singles.tile([1, 6 * 512], dt.float32)
            for t in range(6):
                nc.vector.tensor_copy(outt[:, t * 512:(t + 1) * 512],
                                      pss[t][0:1, :])
            nc.sync.dma_start(out=po[:, :], in_=outt[:])

    nc.compile()
    return nc


def _prep_core(core, kl_logits, jsnm_logits, jsnl_logits, kl_t, jsnm_t, jsnl_t):
    """Per-core flat xb/tb (fp16) arrays, tile-major [128, ncols, Q] blocks."""
    lo, hi = core * NCORE, (core + 1) * NCORE
    cols = np.zeros((NCOLS, NPAD), dtype=np.float16)
    cols[0:4, :NCORE] = kl_logits[lo:hi].T
    cols[4:7, :NCORE] = jsnm_logits[lo:hi].T
    cols[7:10, :NCORE] = jsnl_logits[lo:hi].T
    tg = np.zeros((3, NPAD), dtype=np.float16)
    tg[0, :NCORE] = kl_t[lo:hi]
    tg[0, NCORE:] = 7.0     # padding: no class matches -> weight 0
    tg[1, :NCORE] = jsnm_t[lo:hi]
    tg[2, :NCORE] = jsnl_t[lo:hi]

    carr = cols.reshape(NCOLS, 128, RPP).transpose(1, 0, 2)
    tarr = tg.reshape(3, 128, RPP).transpose(1, 0, 2)
    xparts, tparts = [], []
    off = 0
    for Q in TILE_Q:
        xparts.append(np.ascontiguousarray(carr[:, :, off:off + Q]).ravel())
        tparts.append(np.ascontiguousarray(tarr[:, :, off:off + Q]).ravel())
        off += Q
    return np.concatenate(xparts), np.concatenate(tparts)


def kernel(kl_logits, jsnm_logits, jsnl_logits, class_weights, kl_t,
           jsnm_t, jsnl_t):
    kl_logits = np.asarray(kl_logits, dtype=np.float32)
    jsnm_logits = np.asarray(jsnm_logits, dtype=np.float32)
    jsnl_logits = np.asarray(jsnl_logits, dtype=np.float32)
    class_weights = np.asarray(class_weights, dtype=np.float32)
    kl_t = np.asarray(kl_t).astype(np.int32)
    jsnm_t = np.asarray(jsnm_t).astype(np.int32)
    jsnl_t = np.asarray(jsnl_t).astype(np.int32)

    _ensure_actroot()
    os.environ["BASS_ACT_ROOT_JSON_PATH"] = os.path.join(
        ACTROOT, "act_info.json")

    if "nc" not in _CACHED:
        _CACHED["nc"] = _build_nc()
    nc = _CACHED["nc"]

    cwm = np.zeros((1, 5), dtype=np.float32)
    cwm[0, :] = class_weights
    in_maps = []
    for core in range(NCORES):
        xbv, tbv = _prep_core(core, kl_logits, jsnm_logits, jsnl_logits,
                              kl_t, jsnm_t, jsnl_t)
        in_maps.append({"xb": xbv, "tb": tbv, "cw": cwm})

    res = run_bass_kernel_spmd(nc, in_maps, core_ids=list(range(NCORES)),
                               trace=False)

    S = np.zeros(3, dtype=np.float64)
    for core in range(NCORES):
        p = res.results[core]["po"].astype(np.float64).reshape(6, 512)
        ps = p.sum(axis=1)
        S += 0.5 * (ps[0:3] + ps[3:6])

    l_kl = S[0] / (N * 4)
    l_m = S[1] / (N * 3)
    l_l = S[2] / (N * 3)
    total = (l_kl + l_m + l_l) / 3.0
    return (np.float32(total), np.float32(l_kl), np.float32(l_m),
            np.float32(l_l))


# revision 13
# speedup vs baseline: 6.8384x; 6.8384x over previous
"""CORAL focal multi-task loss on 8 Trainium2 NeuronCores.

Data-parallel: the 2M-row batch is split into 8 shards of 250k rows
(padded to 128*1954). Each core computes PSUM partial sums of the
weighted focal-CORAL loss elements for the 3 tasks; the host sums the
8 x 3 x 512 partials and normalizes.

Math. For one element with logit x, ordinal bit b = (t > c), kl weight w:
  loss_elem = w * (0.75 - 0.5 b) * Fc((1-2b) * x),
  Fc(z) = sigmoid(z)^2 * softplus(z)
since  -log(sigmoid(z)) = softplus(-z)  and  1 - sigmoid(z) = sigmoid(-z).
Fc is evaluated in ONE ScalarE pass via a custom activation table (the
`gelu` slot of the gelu_and_others set is rewritten with Taylor cubics of
Fc at the stock bucket centers; see _ensure_actroot / work/mktable.py).

Device pipeline per tile ([128, 10*Q] fp16 column slabs):
  DVE: smh = (t <= c) - 0.5            (per-column tensor_scalar, = +-0.5)
       y'  = x * smh                    (= +-x/2)
       a   = ACT Fc(2*y')               (ScalarE, custom table)
       wa  = a * w_row (broadcast AP);  q = smh * wa
  PE:  ones^T @ {wa, q} chunks accumulate into 6 PSUM [1,512] tiles;
       since (0.75 - 0.5 b) = 0.5 + 0.5*(2*smh), per-task
       S = 0.5 * (sum(wa) + sum(q))  (combined on host).
  w_row = sum_c (kl_t == c) * cw[c]; padding rows carry kl_t = 7 -> w = 0.
"""

import json
import os
import shutil
import numpy as np

import concourse.bacc as bacc
import concourse.mybir as mybir
import concourse.tile as tile
from concourse.bass_utils import run_bass_kernel_spmd

AluOp = mybir.AluOpType
ActFn = mybir.ActivationFunctionType
dt = mybir.dt

EPS = 1e-8
N = 2_000_000
NCORES = 8
NCORE = N // NCORES            # 250_000 rows per core
RPP = 1954                     # rows per partition (128*1954 = 250_112)
NPAD = 128 * RPP               # padded rows per core
TILE_Q = [490, 490, 490, 484]  # rows-per-partition per tile (sum = RPP)
NCOLS = 10                     # 4 kl + 3 jsnm + 3 jsnl
TASKS = [(0, 4), (4, 3), (7, 3)]  # (column offset, n columns)

def _actroot_dir():
    base = os.path.dirname(os.path.abspath(__file__))
    cand = os.path.join(base, "actroot")
    try:
        os.makedirs(cand, exist_ok=True)
        probe = os.path.join(cand, ".w")
        open(probe, "w").write("x")
        os.remove(probe)
        return cand
    except OSError:
        import tempfile
        return os.path.join(tempfile.gettempdir(), "coral_actroot")


ACTROOT = _actroot_dir()

_CACHED = {}


# ---------------------------------------------------------------------------
# Custom activation table: rewrite the `gelu` buckets of gelu_and_others so
# that ActivationFunctionType.Gelu evaluates Fc(z) = sigmoid(z)^2*softplus(z).
# Bucket entry format (32B): [d0, d1, d2, d3, x0, 0, 0, 0] — Taylor coeffs
# around x0. Entries 0..503 are gelu's dense buckets, 504/505 small-signal,
# 506 positive saturation, 507 negative saturation.
# ---------------------------------------------------------------------------

def _fc_taylor_coeffs(x0s):
    """Taylor coefficients [F, F', F''/2, F'''/6] of Fc at each x0 (float64)."""
    x = np.asarray(x0s, dtype=np.float64)
    u = 1.0 / (1.0 + np.exp(-x))
    sp = np.logaddexp(0.0, x)
    up = u * (1 - u)
    F = u * u * sp
    A = 2 * (1 - u) * sp + u
    F1 = u * u * A
    Ap = up * (1 - 2 * sp) + 2 * u * (1 - u)
    F2 = 2 * u * up * A + u * u * Ap
    # third derivative numerically (Richardson) — plenty accurate in f64
    h = 1e-4

    def F2f(xx):
        uu = 1.0 / (1.0 + np.exp(-xx))
        ssp = np.logaddexp(0.0, xx)
        uup = uu * (1 - uu)
        AA = 2 * (1 - uu) * ssp + uu
        AAp = uup * (1 - 2 * ssp) + 2 * uu * (1 - uu)
        return 2 * uu * uup * AA + uu * uu * AAp

    F3 = (F2f(x + h) - F2f(x - h)) / (2 * h)
    return F, F1, F2 / 2.0, F3 / 6.0


def _ensure_actroot():
    """Build ACTROOT (idempotent) from the stock pwp_bin_trainium dir."""
    marker = os.path.join(ACTROOT, ".fc_table_v2")
    if os.path.exists(marker):
        return
    from neuronxcc.driver.Job import Job
    from neuronxcc.driver.jobs.support.FindActInfo import findActInfoFile

    src = os.path.dirname(findActInfoFile(Job.getPackageDir(), "gen3"))
    os.makedirs(ACTROOT, exist_ok=True)
    for f in os.listdir(src):
        shutil.copy(os.path.join(src, f), os.path.join(ACTROOT, f))

    bkt_path = os.path.join(ACTROOT, "gelu_and_others_bkt.bin")
    e = np.frombuffer(open(bkt_path, "rb").read(),
                      dtype=np.float32).reshape(-1, 8).copy()
    x0 = e[:504, 4].astype(np.float64)
    d0, d1, d2, d3 = _fc_taylor_coeffs(x0)
    e[:504, 0] = d0
    e[:504, 1] = d1
    e[:504, 2] = d2
    e[:504, 3] = d3
    t0 = [c[0] for c in _fc_taylor_coeffs(np.array([0.0]))]
    e[504, 0:4] = t0
    e[504, 4] = 0.0
    e[505, 0:4] = t0
    e[505, 4] = 0.0
    # positive tail (z > ~4.918): cubic least-squares fit around 0
    zs = np.linspace(4.9185, 10.0, 400)
    u = 1.0 / (1.0 + np.exp(-zs))
    ys = u * u * np.logaddexp(0.0, zs)
    A = np.vstack([np.ones_like(zs), zs, zs**2, zs**3]).T
    c, *_ = np.linalg.lstsq(A, ys, rcond=None)
    e[506, 0:4] = c
    e[506, 4] = 0.0
    e[507, 0:4] = 0.0
    e[507, 4] = 0.0
    open(bkt_path, "wb").write(e.tobytes())

    pj_path = os.path.join(ACTROOT, "gelu_and_others.json")
    pj = json.load(open(pj_path))
    fz = int(np.float32(float(d0[0]) * 0 + 0.17328679513998632).view(np.uint32))
    for ent in pj["profile_meta_data"]:
        if ent["func_name"] == "gelu_4p":
            ent["fzero_result"] = fz
            ent["fpinf_result"] = int(np.float32(3.4028235e38).view(np.uint32))
            ent["fninf_result"] = 0
    json.dump(pj, open(pj_path, "w"), indent=1)
    open(marker, "w").write("ok")


def _build_nc(rep=1):
    nc = bacc.Bacc("TRN2", num_devices=NCORES)

    xb = nc.dram_tensor("xb", [NPAD * NCOLS], dt.float16, kind="ExternalInput")
    tb = nc.dram_tensor("tb", [NPAD * 3], dt.float16, kind="ExternalInput")
    cw = nc.dram_tensor("cw", [1, 5], dt.float32, kind="ExternalInput")
    po = nc.dram_tensor("po", [1, 6 * 512], dt.float32, kind="ExternalOutput")

    with tile.TileContext(nc) as tc:
        with (
            tc.tile_pool(name="singles", bufs=1) as singles,
            tc.tile_pool(name="io", bufs=3) as io,
            tc.tile_pool(name="work", bufs=2) as wk,
            tc.tile_pool(name="ps", bufs=1, space="PSUM") as psp,
        ):
            cwt = singles.tile([128, 5], dt.float32)
            nc.sync.dma_start(out=cwt[:], in_=cw[:].to_broadcast((128, 5)))
            ones16 = singles.tile([128, 1], dt.float16)
            nc.vector.memset(ones16[:], 1.0)

            pss = [psp.tile([1, 512], dt.float32, tag=f"ps{t}", name=f"ps{t}")
                   for t in range(6)]
            started = [False] * 6

            # total matmul chunks per task (to set stop on the last one)
            def chunks_for(Q, C):
                L = C * Q
                out = []
                off = 0
                while off < L:
                    ln = min(512, L - off)
                    out.append((off, ln))
                    off += ln
                return out

            total_mm = [sum(len(chunks_for(Q, C)) for Q in TILE_Q)
                        for _, C in TASKS] * 2
            done_mm = [0] * 6

            import contextlib
            loop_ctx = (tc.For_i(0, rep, 1, hint_engines=(
                mybir.EngineType.DVE, mybir.EngineType.Activation,
                mybir.EngineType.SP, mybir.EngineType.PE)) if rep > 1
                else contextlib.nullcontext())
            with loop_ctx:
                base_x = 0
                base_t = 0
                for j, Q in enumerate(TILE_Q):
                    F = NCOLS * Q
                    xt = io.tile([128, F], dt.float16, tag="xt")
                    nc.sync.dma_start(
                        out=xt[:],
                        in_=xb[base_x:base_x + 128 * F].rearrange(
                            "(p f) -> p f", p=128))
                    tt = io.tile([128, 3 * Q], dt.float16, tag="tt")
                    nc.sync.dma_start(
                        out=tt[:],
                        in_=tb[base_t:base_t + 128 * 3 * Q].rearrange(
                            "(p f) -> p f", p=128))
                    base_x += 128 * F
                    base_t += 128 * 3 * Q

                    # smh = (t <= c) - 0.5   (per ordinal column)
                    smh = wk.tile([128, F], dt.float16, tag="smh")
                    ts_eng = nc.gpsimd if os.environ.get("KGPS") else nc.vector
                    col = 0
                    for tsk, (coff, C) in enumerate(TASKS):
                        tslab = tt[:, tsk * Q:(tsk + 1) * Q]
                        for c in range(C):
                            ts_eng.tensor_scalar(
                                smh[:, col * Q:(col + 1) * Q], tslab,
                                float(c), 0.5, AluOp.is_le, AluOp.subtract)
                            col += 1

                    # y' = x * smh ; a = Fc(2 y')
                    yt = wk.tile([128, F], dt.float16, tag="yt")
                    nc.vector.tensor_mul(yt[:], xt[:], smh[:])
                    at = wk.tile([128, F], dt.float16, tag="at")
                    nc.scalar.activation(at[:], yt[:], ActFn.Gelu, scale=2.0)

                    # per-row weight from kl_t
                    wr = wk.tile([128, Q], dt.float16, tag="wr")
                    tmpw = wk.tile([128, Q], dt.float16, tag="tmpw")
                    tkl = tt[:, 0:Q]
                    for c in range(5):
                        dst = wr if c == 0 else tmpw
                        nc.vector.tensor_scalar(
                            dst[:], tkl, float(c), cwt[:, c:c + 1],
                            AluOp.is_equal, AluOp.mult)
                        if c > 0:
                            nc.vector.tensor_add(wr[:], wr[:], tmpw[:])

                    # wa = a * w_row ; q = smh * wa
                    # sum(ab*w*a) = 0.5*sum(wa) + 0.5*sum(q)
                    wa = wk.tile([128, F], dt.float16, tag="wa")
                    wr_b = wr[:].rearrange("p (o q) -> p o q", o=1).to_broadcast(
                        (128, NCOLS, Q))
                    nc.vector.tensor_tensor(
                        wa[:].rearrange("p (o q) -> p o q", o=NCOLS),
                        at[:].rearrange("p (o q) -> p o q", o=NCOLS),
                        wr_b, AluOp.mult)
                    qt = wk.tile([128, F], dt.float16, tag="qt")
                    nc.vector.tensor_mul(qt[:], smh[:], wa[:])

                    # PE reduction: ones^T @ {wa, q} chunks -> PSUM accumulate
                    for half, src in ((0, wa), (1, qt)):
                        for t, (coff, C) in enumerate(TASKS):
                            k = half * 3 + t
                            for (off, ln) in chunks_for(Q, C):
                                first = not started[k]
                                done_mm[k] += 1
                                last = done_mm[k] == total_mm[k]
                                nc.tensor.matmul(
                                    pss[k][0:1, 0:ln], ones16[:, 0:1],
                                    src[:, coff * Q + off: coff * Q + off + ln],
                                    start=first, stop=last)
                                started[k] = True

            outt = singles.tile([1, 6 * 512], dt.float32)
            for t in range(6):
                nc.vector.tensor_copy(outt[:, t * 512:(t + 1) * 512],
                                      pss[t][0:1, :])
            nc.sync.dma_start(out=po[:, :], in_=outt[:])

    nc.compile()
    return nc


def _prep_core(core, kl_logits, jsnm_logits, jsnl_logits, kl_t, jsnm_t, jsnl_t):
    """Per-core flat xb/tb (fp16) arrays, tile-major [128, ncols, Q] blocks."""
    lo, hi = core * NCORE, (core + 1) * NCORE
    cols = np.zeros((NCOLS, NPAD), dtype=np.float16)
    cols[0:4, :NCORE] = kl_logits[lo:hi].T
    cols[4:7, :NCORE] = jsnm_logits[lo:hi].T
    cols[7:10, :NCORE] = jsnl_logits[lo:hi].T
    tg = np.zeros((3, NPAD), dtype=np.float16)
    tg[0, :NCORE] = kl_t[lo:hi]
    tg[0, NCORE:] = 7.0     # padding: no class matches -> weight 0
    tg[1, :NCORE] = jsnm_t[lo:hi]
    tg[2, :NCORE] = jsnl_t[lo:hi]

    carr = cols.reshape(NCOLS, 128, RPP).transpose(1, 0, 2)
    tarr = tg.reshape(3, 128, RPP).transpose(1, 0, 2)
    xparts, tparts = [], []
    off = 0
    for Q in TILE_Q:
        xparts.append(np.ascontiguousarray(carr[:, :, off:off + Q]).ravel())
        tparts.append(np.ascontiguousarray(tarr[:, :, off:off + Q]).ravel())
        off += Q
    return np.concatenate(xparts), np.concatenate(tparts)


def kernel(kl_logits, jsnm_logits, jsnl_logits, class_weights, kl_t,
           jsnm_t, jsnl_t):
    kl_logits = np.asarray(kl_logits, dtype=np.float32)
    jsnm_logits = np.asarray(jsnm_logits, dtype=np.float32)
    jsnl_logits = np.asarray(jsnl_logits, dtype=np.float32)
    class_weights = np.asarray(class_weights, dtype=np.float32)
    kl_t = np.asarray(kl_t).astype(np.int32)
    jsnm_t = np.asarray(jsnm_t).astype(np.int32)
    jsnl_t = np.asarray(jsnl_t).astype(np.int32)

    _ensure_actroot()
    os.environ["BASS_ACT_ROOT_JSON_PATH"] = os.path.join(
        ACTROOT, "act_info.json")

    if "nc" not in _CACHED:
        _CACHED["nc"] = _build_nc()
    nc = _CACHED["nc"]

    cwm = np.zeros((1, 5), dtype=np.float32)
    cwm[0, :] = class_weights
    in_maps = []
    for core in range(NCORES):
        xbv, tbv = _prep_core(core, kl_logits, jsnm_logits, jsnl_logits,
                              kl_t, jsnm_t, jsnl_t)
        in_maps.append({"xb": xbv, "tb": tbv, "cw": cwm})

    res = run_bass_kernel_spmd(nc, in_maps, core_ids=list(range(NCORES)),
                               trace=False)

    S = np.zeros(3, dtype=np.float64)
    for core in range(NCORES):
        p = res.results[core]["po"].astype(np.float64).reshape(6, 512)
        ps = p.sum(axis=1)
        S += 0.5 * (ps[0:3] + ps[3:6])

    l_kl = S[0] / (N * 4)
    l_m = S[1] / (N * 3)
    l_l = S[2] / (N * 3)
    total = (l_kl + l_m + l_l) / 3.0
    return (np.float32(total), np.float32(l_kl), np.float32(l_m),
            np.float32(l_l))
